# revision 1
# baseline (speedup 1.0000x reference)
"""Trainium2 Bass kernel for nn_Attention_19971597927194 (GNN message passing).

Destination-sharded, zero-collective design:
  - Edges sorted by destination i0; each of 8 cores owns 12500 consecutive
    destination nodes and every edge pointing into them. No cross-core
    reduction is needed.
  - Host does layout-only prep (concat / sort / index packing; no arithmetic
    on tensor values).
  - Destinations are packed into blocks of <=128 dests. Per block, edges are
    gathered via dma_gather (SWDGE MoE primitive): the per-edge source rows
    come from a [k|eigs|1|v] table (1280B rows) split in 4 chunks of 25000
    rows (int16 index limit), the per-edge destination rows from a local
    [q|eigs] table (768B rows). Scores, exp-weights, and a fused
    is_equal*weight one-hot mask (tensor_scalar) feed PE matmuls that
    accumulate [den|num] per destination in PSUM. Softmax normalization
    commutes with the segment sum, so it is applied once per destination at
    the end. Output rows are scattered with an indirect DMA.
"""
import sys

sys.path.insert(0, '/opt/trn_rl_repo')

import numpy as np

N_NODES = 100000
N_EDGES = 1000000
HID = 128
EDIM = 32
N_PATH = 6
NCORES = 8
P = 128

RU = 192                  # U row: [q(128) | eigs(32) | pad(32)] = 768B
RG = 320                  # G row: [k(128) | eigs(32) | 1 | v(128) | pad(31)] = 1280B
ONES_COL = HID + EDIM     # 160: the constant-1 column in G
NCHUNK = 4                # G table split (int16 index limit)
TC = 3                    # tiles per chunk group per block
T = NCHUNK * TC           # 12 tiles (of 128 slots) per block
CAPC = TC * P             # 384: per-chunk slot capacity per block
BB = 2                    # blocks per gather batch
D_CORE = N_NODES // NCORES
CHUNK = N_NODES // NCHUNK  # 25000

_INVSQRT = float(1.0 / np.sqrt(np.float32(HID)))


def _wrap_idx(lst):
    """dma_gather index layout: [128, n/16] = wrapped [16, n/16], tiled x8."""
    n = lst.shape[0]
    assert n % 16 == 0
    w = lst.reshape(n // 16, 16).T
    return np.tile(w, (8, 1)).astype(np.int16)


def _prepare(q, k, v, eigs, lambda0, path_emb, indices, path_type):
    q = np.asarray(q, dtype=np.float32)
    k = np.asarray(k, dtype=np.float32)
    v = np.asarray(v, dtype=np.float32)
    eigs = np.asarray(eigs, dtype=np.float32)
    lam0 = np.asarray(lambda0, dtype=np.float32).reshape(1, 1)
    pemb = np.asarray(path_emb, dtype=np.float32).reshape(1, N_PATH)
    i0 = np.asarray(indices[0])
    i1 = np.asarray(indices[1])
    pt = np.asarray(path_type)

    Gt = np.zeros((N_NODES, RG), dtype=np.float32)
    Gt[:, 0:HID] = k
    Gt[:, HID:HID + EDIM] = eigs
    Gt[:, ONES_COL] = 1.0
    Gt[:, ONES_COL + 1:ONES_COL + 1 + HID] = v
    Ut = np.zeros((N_NODES, RU), dtype=np.float32)
    Ut[:, 0:HID] = q
    Ut[:, HID:HID + EDIM] = eigs

    order = np.argsort(i0, kind='stable')
    i0s = i0[order].astype(np.int64)
    i1s = i1[order].astype(np.int64)
    pts = pt[order].astype(np.int64)
    core_bounds = np.searchsorted(i0s, np.arange(NCORES + 1) * D_CORE)
    eye = np.eye(N_PATH, dtype=np.float32)

    # ---- per-core block packing (consecutive dests; per-chunk slot quota) ----
    core_blocks = []
    for c in range(NCORES):
        lo, hi = core_bounds[c], core_bounds[c + 1]
        i0l = i0s[lo:hi] - c * D_CORE
        chl = (i1s[lo:hi] // CHUNK).astype(np.int64)
        deg_pc = np.zeros((D_CORE, NCHUNK), np.int64)
        np.add.at(deg_pc, (i0l, chl), 1)
        assert deg_pc.max() <= CAPC
        blocks = []
        ds = 0
        while ds < D_CORE:
            de = ds
            cnt = np.zeros(NCHUNK, np.int64)
            while de < D_CORE and de - ds < P and np.all(cnt + deg_pc[de] <= CAPC):
                cnt += deg_pc[de]
                de += 1
            assert de > ds
            blocks.append((ds, de))
            ds = de
        core_blocks.append((lo, hi, blocks))
    B = max(len(b) for _, _, b in core_blocks)
    if B % BB:
        B += BB - (B % BB)
    NBATCH = B // BB

    per_core = []
    for c in range(NCORES):
        lo, hi, blocks = core_blocks[c]
        i0l = i0s[lo:hi] - c * D_CORE
        i1c = i1s[lo:hi]
        ptc = pts[lo:hi]
        csum = np.concatenate([[0], np.cumsum(np.bincount(i0l, minlength=D_CORE))])

        gi16 = np.zeros((NBATCH, NCHUNK, 128, BB * TC * P // 16), dtype=np.int16)
        ui16 = np.zeros((NBATCH, 128, BB * T * P // 16), dtype=np.int16)
        dloc = np.full((B, P, T), 1.0e9, dtype=np.float32)
        oh = np.zeros((B, P, T, N_PATH), dtype=np.float32)
        scat = np.full((B, P), 1 << 20, dtype=np.int32)

        gl_lists = np.zeros((NBATCH, NCHUNK, BB * TC * P), dtype=np.int64)
        ul_lists = np.zeros((NBATCH, BB * T * P), dtype=np.int64)

        for b, (ds, de) in enumerate(blocks):
            bt, bb = divmod(b, BB)
            e0, e1 = csum[ds], csum[de]
            sl = slice(e0, e1)
            ii0 = i0l[sl]
            ii1 = i1c[sl]
            ipt = ptc[sl]
            ch = ii1 // CHUNK
            # order edges by chunk, stable
            o2 = np.argsort(ch, kind='stable')
            ii0, ii1, ipt, ch = ii0[o2], ii1[o2], ipt[o2], ch[o2]
            cc = np.concatenate([[0], np.cumsum(np.bincount(ch, minlength=NCHUNK))])
            for cch in range(NCHUNK):
                g0, g1 = cc[cch], cc[cch + 1]
                n_g = g1 - g0
                assert n_g <= CAPC
                j = np.arange(n_g)
                kk = j // P          # tile within chunk group (0..TC-1)
                pp = j % P
                # block-tile index t = cch*TC + kk ; slot in per-chunk idx
                # list for this batch: bb*CAPC + j
                dloc[b, pp, cch * TC + kk] = (ii0[g0:g1] - ds).astype(np.float32)
                oh[b, pp, cch * TC + kk, :] = eye[ipt[g0:g1]]
                gl_lists[bt, cch, bb * CAPC + j] = ii1[g0:g1] - cch * CHUNK
                # u index follows staging layout: staging tile s = cch*(BB*TC)
                # + bb*TC + kk, u-list position s*P + pp
                s_t = cch * (BB * TC) + bb * TC + kk
                ul_lists[bt, s_t * P + pp] = ii0[g0:g1]
            scat[b, 0:de - ds] = np.arange(ds, de, dtype=np.int32)

        for bt in range(NBATCH):
            for cch in range(NCHUNK):
                gi16[bt, cch] = _wrap_idx(gl_lists[bt, cch])
            ui16[bt] = _wrap_idx(ul_lists[bt])

        per_core.append(dict(
            G=Gt, U=Ut[c * D_CORE:(c + 1) * D_CORE],
            gi16=gi16.reshape(NBATCH, NCHUNK * 128, BB * TC * P // 16),
            ui16=ui16,
            dloc=dloc, oh=oh.reshape(B, P, T * N_PATH),
            scat=scat.transpose(1, 0).copy(),
            lam0=lam0, pemb=pemb,
        ))
    return per_core, B


def _build_bass(B):
    import concourse.bass as bass
    import concourse.bacc as bacc
    import concourse.mybir as mybir
    from concourse.tile import TileContext

    dt = mybir.dt
    Alu = mybir.AluOpType
    Act = mybir.ActivationFunctionType
    NBATCH = B // BB
    NIG = BB * TC * P          # idxs per chunk gather call (768)
    NIU = BB * T * P           # idxs per U gather call (3072)

    nc = bacc.Bacc(None)
    G = nc.declare_dram_parameter("G", [N_NODES, RG], dt.float32, isOutput=False)
    U = nc.declare_dram_parameter("U", [D_CORE, RU], dt.float32, isOutput=False)
    gi16 = nc.declare_dram_parameter("gi16", [NBATCH, NCHUNK * 128, NIG // 16],
                                     dt.int16, isOutput=False)
    ui16 = nc.declare_dram_parameter("ui16", [NBATCH, 128, NIU // 16],
                                     dt.int16, isOutput=False)
    dloc = nc.declare_dram_parameter("dloc", [B, P, T], dt.float32, isOutput=False)
    oh = nc.declare_dram_parameter("oh", [B, P, T * N_PATH], dt.float32,
                                   isOutput=False)
    scat = nc.declare_dram_parameter("scat", [P, B], dt.int32, isOutput=False)
    lam0 = nc.declare_dram_parameter("lam0", [1, 1], dt.float32, isOutput=False)
    pemb = nc.declare_dram_parameter("pemb", [1, N_PATH], dt.float32, isOutput=False)
    out = nc.declare_dram_parameter("out", [D_CORE, HID], dt.float32, isOutput=True)

    with TileContext(nc) as tc:
        with tc.tile_pool(name="const", bufs=1) as cpool, \
             tc.tile_pool(name="gath", bufs=2) as gpool, \
             tc.tile_pool(name="work", bufs=2) as wpool, \
             tc.tile_pool(name="small", bufs=3) as spool, \
             tc.tile_pool(name="psum", bufs=2, space="PSUM") as pspool, \
             tc.tile_pool(name="psc", bufs=1, space="PSUM") as pscpool:

            iota = cpool.tile([P, P], dt.float32)
            nc.gpsimd.iota(iota[:], pattern=[[1, P]], base=0, channel_multiplier=0,
                           allow_small_or_imprecise_dtypes=True)
            ones = cpool.tile([1, P], dt.float32)
            nc.vector.memset(ones[:], 1.0)
            scat_all = cpool.tile([P, B], dt.int32)
            nc.sync.dma_start(out=scat_all[:], in_=scat[:])

            tl = cpool.tile([1, 1], dt.float32)
            nc.sync.dma_start(out=tl[:], in_=lam0[:])
            tle = cpool.tile([1, 1], dt.float32)
            nc.scalar.activation(out=tle[:], in_=tl[:], func=Act.Exp)
            lps = pscpool.tile([P, 1], dt.float32)
            nc.tensor.matmul(out=lps[:], lhsT=ones[:], rhs=tle[:], start=True,
                             stop=True)
            lamb = cpool.tile([P, 1], dt.float32)
            nc.vector.tensor_copy(out=lamb[:], in_=lps[:])

            tp = cpool.tile([1, N_PATH], dt.float32)
            nc.sync.dma_start(out=tp[:], in_=pemb[:])
            tpe = cpool.tile([1, N_PATH], dt.float32)
            nc.scalar.activation(out=tpe[:], in_=tp[:], func=Act.Exp)
            nc.vector.tensor_scalar(out=tpe[:], in0=tpe[:], scalar1=5.0,
                                    scalar2=None, op0=Alu.min)
            pps = pscpool.tile([P, N_PATH], dt.float32)
            nc.tensor.matmul(out=pps[:], lhsT=ones[:], rhs=tpe[:], start=True,
                             stop=True)
            w1tab = cpool.tile([P, N_PATH], dt.float32)
            nc.vector.tensor_copy(out=w1tab[:], in_=pps[:])
            w1ap = w1tab[:]
            w1bc = bass.AP(w1ap.tensor, w1ap.offset,
                           [w1ap.ap[0], [0, T], w1ap.ap[1]])

            for bt in range(NBATCH):
                # ---- gathers for this batch (BB blocks) ----
                Gg = gpool.tile([P, BB * T, RG], dt.float32, tag="Gg")
                Ug = gpool.tile([P, BB * T, RU], dt.float32, tag="Ug")
                uit = spool.tile([P, NIU // 16], dt.int16, tag="uit")
                nc.sync.dma_start(out=uit[:], in_=ui16[bt])
                nc.gpsimd.dma_gather(Ug[:], U[:], uit[:], NIU, NIU, RU,
                                     single_packet=False)
                for cch in range(NCHUNK):
                    git = spool.tile([P, NIG // 16], dt.int16, tag=f"git{cch}")
                    nc.sync.dma_start(
                        out=git[:],
                        in_=gi16[bt, cch * 128:(cch + 1) * 128, :])
                    dst = Gg[:, cch * (BB * TC):(cch + 1) * (BB * TC), :]
                    nc.gpsimd.dma_gather(dst, G[cch * CHUNK:(cch + 1) * CHUNK, :],
                                         git[:], NIG, NIG, RG,
                                         single_packet=False)

                for bb in range(BB):
                    b = bt * BB + bb
                    # staging tile for block-tile t=(cch*TC+kk):
                    #   s(t) = cch*(BB*TC) + bb*TC + kk
                    def gsl(t):
                        cch, kk = divmod(t, TC)
                        return cch * (BB * TC) + bb * TC + kk

                    dlc = spool.tile([P, T], dt.float32, tag="dlc")
                    nc.sync.dma_start(out=dlc[:], in_=dloc[b])
                    oht = spool.tile([P, T, N_PATH], dt.float32, tag="oht")
                    nc.sync.dma_start(out=oht[:], in_=oh[b])

                    # 4-D views of this block's staging tiles:
                    # [P, cch(4), kk(TC), r]
                    Gap = Gg[:]
                    Gv = bass.AP(Gap.tensor, Gap.offset + bb * TC * RG,
                                 [Gap.ap[0], [BB * TC * RG, NCHUNK],
                                  [RG, TC], [1, RG]])
                    Uap = Ug[:]
                    Uv = bass.AP(Uap.tensor, Uap.offset + bb * TC * RU,
                                 [Uap.ap[0], [BB * TC * RU, NCHUNK],
                                  [RU, TC], [1, RU]])

                    RS = HID + EDIM  # 160: real score width
                    prod = wpool.tile([P, T, RS], dt.float32, tag="prod")
                    prod4 = prod[:].rearrange("p (c k) r -> p c k r", c=NCHUNK)
                    nc.vector.tensor_tensor(out=prod4, in0=Uv[:, :, :, 0:RS],
                                            in1=Gv[:, :, :, 0:RS], op=Alu.mult)
                    red1 = spool.tile([P, T], dt.float32, tag="red1")
                    nc.vector.tensor_reduce(
                        out=red1[:], in_=prod[:, :, 0:HID],
                        axis=mybir.AxisListType.X, op=Alu.add)
                    red2 = spool.tile([P, T], dt.float32, tag="red2")
                    nc.vector.tensor_reduce(
                        out=red2[:], in_=prod[:, :, HID:HID + EDIM],
                        axis=mybir.AxisListType.X, op=Alu.add)
                    score = spool.tile([P, T], dt.float32, tag="score")
                    nc.vector.tensor_scalar(out=score[:], in0=red1[:],
                                            scalar1=_INVSQRT, scalar2=None,
                                            op0=Alu.mult)
                    nc.vector.scalar_tensor_tensor(out=score[:], in0=red2[:],
                                                   scalar=lamb[:], in1=score[:],
                                                   op0=Alu.mult, op1=Alu.add)
                    nc.vector.tensor_scalar(out=score[:], in0=score[:],
                                            scalar1=2.0, scalar2=None,
                                            op0=Alu.min)
                    w0 = spool.tile([P, T], dt.float32, tag="w0")
                    nc.scalar.activation(out=w0[:], in_=score[:], func=Act.Exp)
                    nc.vector.tensor_scalar(out=w0[:], in0=w0[:], scalar1=5.0,
                                            scalar2=None, op0=Alu.min)
                    ohw = wpool.tile([P, T, N_PATH], dt.float32, tag="ohw")
                    nc.vector.tensor_tensor(out=ohw[:], in0=oht[:], in1=w1bc,
                                            op=Alu.mult)
                    w1 = spool.tile([P, T], dt.float32, tag="w1")
                    nc.vector.tensor_reduce(out=w1[:], in_=ohw[:],
                                            axis=mybir.AxisListType.X,
                                            op=Alu.add)

                    mw0 = wpool.tile([P, T, P], dt.float32, tag="mw0")
                    mw1 = wpool.tile([P, T, P], dt.float32, tag="mw1")
                    for t in range(T):
                        nc.vector.tensor_scalar(out=mw0[:, t, :], in0=iota[:],
                                                scalar1=dlc[:, t:t + 1],
                                                scalar2=w0[:, t:t + 1],
                                                op0=Alu.is_equal, op1=Alu.mult)
                        nc.vector.tensor_scalar(out=mw1[:, t, :], in0=iota[:],
                                                scalar1=dlc[:, t:t + 1],
                                                scalar2=w1[:, t:t + 1],
                                                op0=Alu.is_equal, op1=Alu.mult)
                    ps0 = pspool.tile([P, 1 + HID], dt.float32, tag="ps0")
                    ps1 = pspool.tile([P, 1 + HID], dt.float32, tag="ps1")
                    for t in range(T):
                        rhs = Gv[:, t // TC, t % TC, ONES_COL:ONES_COL + 1 + HID]
                        nc.tensor.matmul(out=ps0[:], lhsT=mw0[:, t, :], rhs=rhs,
                                         start=(t == 0), stop=(t == T - 1))
                    for t in range(T):
                        rhs = Gv[:, t // TC, t % TC, ONES_COL:ONES_COL + 1 + HID]
                        nc.tensor.matmul(out=ps1[:], lhsT=mw1[:, t, :], rhs=rhs,
                                         start=(t == 0), stop=(t == T - 1))

                    obuf = wpool.tile([P, HID], dt.float32, tag="obuf")
                    o1 = wpool.tile([P, HID], dt.float32, tag="o1")
                    for ps, dest in ((ps0, obuf), (ps1, o1)):
                        dz = spool.tile([P, 1], dt.float32, tag="dz")
                        nc.vector.tensor_scalar(out=dz[:], in0=ps[:, 0:1],
                                                scalar1=0.0, scalar2=None,
                                                op0=Alu.is_equal)
                        nc.vector.tensor_tensor(out=dz[:], in0=dz[:],
                                                in1=ps[:, 0:1], op=Alu.add)
                        nc.vector.tensor_scalar(out=dz[:], in0=dz[:], scalar1=2.0,
                                                scalar2=None, op0=Alu.mult)
                        rcp = spool.tile([P, 1], dt.float32, tag="rcp")
                        nc.vector.reciprocal(rcp[:], dz[:])
                        nc.scalar.activation(out=dest[:], in_=ps[:, 1:1 + HID],
                                             func=Act.Copy, scale=rcp[:])
                    nc.vector.tensor_tensor(out=obuf[:], in0=obuf[:], in1=o1[:],
                                            op=Alu.add)
                    nc.gpsimd.indirect_dma_start(
                        out=out[:],
                        out_offset=bass.IndirectOffsetOnAxis(
                            ap=scat_all[:, b:b + 1], axis=0),
                        in_=obuf[:], in_offset=None,
                        bounds_check=D_CORE - 1, oob_is_err=False)

    nc.finalize()
    return nc


_CACHE = {}


def _get_nc(B):
    if B not in _CACHE:
        _CACHE[B] = _build_bass(B)
    return _CACHE[B]


def run(inputs, trace=False):
    from concourse.bass_utils import run_bass_kernel_spmd
    per_core, B = _prepare(**inputs)
    nc = _get_nc(B)
    res = run_bass_kernel_spmd(nc, per_core, list(range(NCORES)), trace=trace)
    outs = [np.asarray(res.results[c]["out"]) for c in range(NCORES)]
    full = np.concatenate(outs, axis=0)
    return full, res


def kernel(**inputs):
    full, _ = run(inputs, trace=False)
    return full



# revision 11
# speedup vs baseline: 2.2156x; 2.2156x over previous
"""Trainium2 Bass kernel for nn_Attention_19971597927194 (GNN message passing).

Destination-sharded, fp16, minimal-host-transfer design.

Wall-clock through the axon tunnel is dominated by shipping input buffers to
the device each call (~10 GB/s) plus a fixed dispatch floor; on-device compute
is <1 ms. So the layout is chosen to minimize TOTAL bytes shipped across the
8 cores:
  - Source-node table G = [k|eigs|1|v] in fp16 (384-elem rows, 768B) is
    shipped SHARDED (12500 rows/core) and AllGathered on device into a
    Shared DRAM tensor, instead of shipping the full table 8x replicated.
  - Dest-node table U = [q|eigs] fp16 is shipped unpadded (160 elems) and
    restrided on device to 256-elem rows for the SWDGE gather.
  - Gather index lists are shipped compact ([16, n/16]) and replicated to
    the [128, n/16] wrapped layout on device with a broadcast DMA.
  - The per-slot path-type is shipped as fp16 codes; the per-edge path
    weight w1 = min(exp(path_emb[pt]), 5) is built on device from a 6-entry
    table (computed on device from path_emb).

Compute structure (per core, proven in the fp32 baseline):
  - Edges sorted by destination; each core owns 12500 consecutive dests.
    Dests are packed into blocks of <=128; per block, edges are gathered via
    dma_gather from the G table (4 chunks of 25000 rows; int16 index limit)
    and the U table. Scores and exp-weights feed is_equal*weight one-hot
    mask matmuls that accumulate [den|num] per destination in PSUM; softmax
    normalization is applied once per destination (it commutes with the
    segment sum). Output rows are scattered with an indirect DMA.
"""
import sys

sys.path.insert(0, '/opt/trn_rl_repo')

import numpy as np

N_NODES = 100000
N_EDGES = 1000000
HID = 128
EDIM = 32
N_PATH = 6
NCORES = 8
P = 128

RS = HID + EDIM           # 160: score width ([q|eigs] . [k|eigs])
RU = 256                  # U row: [q(128) | eigs(32) | pad(96)] fp16 = 512B
RUC = 160                 # U row payload (shipped unpadded)
RG = 384                  # G row: [k(128)|eigs(32)|1|v(128)|pad(95)] fp16 = 768B
ONES_COL = HID + EDIM     # 160: the constant-1 column in G
NCHUNK = 4                # G table split (int16 index limit)
TC = 3                    # tiles per chunk group per block
T = NCHUNK * TC           # 12 tiles (of 128 slots) per block
CAPC = TC * P             # 384: per-chunk slot capacity per block
BB = 2                    # blocks per gather batch
D_CORE = N_NODES // NCORES
CHUNK = N_NODES // NCHUNK  # 25000
NIG = BB * TC * P          # idxs per chunk gather call (768)
NIU = BB * T * P           # idxs per U gather call (3072)
SENT = 30000.0             # dloc sentinel for pad slots (never equals 0..127)

_INVSQRT = float(1.0 / np.sqrt(np.float32(HID)))


def _wrap16(lst):
    """Compact dma_gather index layout: [16, n/16] int16 (device replicates
    to the wrapped [128, n/16] layout with a broadcast DMA)."""
    n = lst.shape[0]
    assert n % 16 == 0
    return lst.reshape(n // 16, 16).T.astype(np.int16)


def _prepare(q, k, v, eigs, lambda0, path_emb, indices, path_type):
    q = np.asarray(q, dtype=np.float32)
    k = np.asarray(k, dtype=np.float32)
    v = np.asarray(v, dtype=np.float32)
    eigs = np.asarray(eigs, dtype=np.float32)
    lam0 = np.asarray(lambda0, dtype=np.float32).reshape(1, 1)
    pemb = np.asarray(path_emb, dtype=np.float32).reshape(1, N_PATH)
    i0 = np.asarray(indices[0])
    i1 = np.asarray(indices[1])
    pt = np.asarray(path_type)

    Gt = np.zeros((N_NODES, RG), dtype=np.float16)
    Gt[:, 0:HID] = k.astype(np.float16)
    Gt[:, HID:HID + EDIM] = eigs.astype(np.float16)
    Gt[:, ONES_COL] = np.float16(1.0)
    Gt[:, ONES_COL + 1:ONES_COL + 1 + HID] = v.astype(np.float16)
    Ut = np.zeros((N_NODES, RU), dtype=np.float16)
    Ut[:, 0:HID] = q.astype(np.float16)
    Ut[:, HID:RUC] = eigs.astype(np.float16)

    order = np.argsort(i0, kind='stable')
    i0s = i0[order].astype(np.int64)
    i1s = i1[order].astype(np.int64)
    pts = pt[order].astype(np.int64)
    core_bounds = np.searchsorted(i0s, np.arange(NCORES + 1) * D_CORE)

    # ---- per-core block packing (consecutive dests; per-chunk slot quota) ----
    core_blocks = []
    for c in range(NCORES):
        lo, hi = core_bounds[c], core_bounds[c + 1]
        i0l = i0s[lo:hi] - c * D_CORE
        chl = (i1s[lo:hi] // CHUNK).astype(np.int64)
        deg_pc = np.zeros((D_CORE, NCHUNK), np.int64)
        np.add.at(deg_pc, (i0l, chl), 1)
        assert deg_pc.max() <= CAPC
        blocks = []
        ds = 0
        while ds < D_CORE:
            de = ds
            cnt = np.zeros(NCHUNK, np.int64)
            while de < D_CORE and de - ds < P and np.all(cnt + deg_pc[de] <= CAPC):
                cnt += deg_pc[de]
                de += 1
            assert de > ds
            blocks.append((ds, de))
            ds = de
        core_blocks.append((lo, hi, blocks))
    B = max(len(b) for _, _, b in core_blocks)
    if B % BB:
        B += BB - (B % BB)
    NBATCH = B // BB

    per_core = []
    for c in range(NCORES):
        lo, hi, blocks = core_blocks[c]
        i0l = i0s[lo:hi] - c * D_CORE
        i1c = i1s[lo:hi]
        ptc = pts[lo:hi]
        csum = np.concatenate([[0], np.cumsum(np.bincount(i0l, minlength=D_CORE))])

        gi16 = np.zeros((NBATCH, NCHUNK * 16, NIG // 16), dtype=np.int16)
        ui16 = np.zeros((NBATCH, 16, NIU // 16), dtype=np.int16)
        dloc = np.full((B, P, T), SENT, dtype=np.float16)
        ptl = np.zeros((B, P, T), dtype=np.float16)
        scat = np.full((B, P), 1 << 20, dtype=np.int32)

        gl_lists = np.zeros((NBATCH, NCHUNK, BB * TC * P), dtype=np.int64)
        ul_lists = np.zeros((NBATCH, BB * T * P), dtype=np.int64)

        for b, (ds, de) in enumerate(blocks):
            bt, bb = divmod(b, BB)
            e0, e1 = csum[ds], csum[de]
            sl = slice(e0, e1)
            ii0 = i0l[sl]
            ii1 = i1c[sl]
            ipt = ptc[sl]
            ch = ii1 // CHUNK
            # order edges by chunk, stable
            o2 = np.argsort(ch, kind='stable')
            ii0, ii1, ipt, ch = ii0[o2], ii1[o2], ipt[o2], ch[o2]
            cc = np.concatenate([[0], np.cumsum(np.bincount(ch, minlength=NCHUNK))])
            for cch in range(NCHUNK):
                g0, g1 = cc[cch], cc[cch + 1]
                n_g = g1 - g0
                assert n_g <= CAPC
                j = np.arange(n_g)
                kk = j // P          # tile within chunk group (0..TC-1)
                pp = j % P
                dloc[b, pp, cch * TC + kk] = (ii0[g0:g1] - ds).astype(np.float16)
                ptl[b, pp, cch * TC + kk] = ipt[g0:g1].astype(np.float16)
                gl_lists[bt, cch, bb * CAPC + j] = ii1[g0:g1] - cch * CHUNK
                # u index follows staging layout: staging tile s = cch*(BB*TC)
                # + bb*TC + kk, u-list position s*P + pp
                s_t = cch * (BB * TC) + bb * TC + kk
                ul_lists[bt, s_t * P + pp] = ii0[g0:g1]
            scat[b, 0:de - ds] = np.arange(ds, de, dtype=np.int32)

        for bt in range(NBATCH):
            for cch in range(NCHUNK):
                gi16[bt, cch * 16:(cch + 1) * 16, :] = _wrap16(gl_lists[bt, cch])
            ui16[bt] = _wrap16(ul_lists[bt])

        per_core.append(dict(
            Gsh=Gt[c * D_CORE:(c + 1) * D_CORE],
            Ush=Ut[c * D_CORE:(c + 1) * D_CORE],
            gi16=gi16, ui16=ui16,
            dloc=dloc, ptl=ptl,
            scat=scat.transpose(1, 0).copy(),
            lam0=lam0, pemb=pemb,
        ))
    return per_core, B


def _build_bass(B, n_devices=NCORES):
    import concourse.bass as bass
    import concourse.bacc as bacc
    import concourse.mybir as mybir
    from concourse.tile import TileContext

    dt = mybir.dt
    Alu = mybir.AluOpType
    Act = mybir.ActivationFunctionType
    NBATCH = B // BB

    nc = bacc.Bacc(None, num_devices=n_devices)
    # single-device build (sim validation): Gsh is the FULL table
    g_rows = D_CORE if n_devices > 1 else N_NODES
    Gshp = nc.declare_dram_parameter("Gsh", [g_rows, RG], dt.float16,
                                     isOutput=False)
    Ushp = nc.declare_dram_parameter("Ush", [D_CORE, RU], dt.float16,
                                     isOutput=False)
    gi16 = nc.declare_dram_parameter("gi16", [NBATCH, NCHUNK * 16, NIG // 16],
                                     dt.int16, isOutput=False)
    ui16 = nc.declare_dram_parameter("ui16", [NBATCH, 16, NIU // 16],
                                     dt.int16, isOutput=False)
    dloc = nc.declare_dram_parameter("dloc", [B, P, T], dt.float16,
                                     isOutput=False)
    ptl = nc.declare_dram_parameter("ptl", [B, P, T], dt.float16,
                                    isOutput=False)
    scat = nc.declare_dram_parameter("scat", [P, B], dt.int32, isOutput=False)
    lam0 = nc.declare_dram_parameter("lam0", [1, 1], dt.float32, isOutput=False)
    pemb = nc.declare_dram_parameter("pemb", [1, N_PATH], dt.float32,
                                     isOutput=False)
    out = nc.declare_dram_parameter("out", [D_CORE, HID], dt.float32,
                                    isOutput=True)

    def bcast8(src_ap):
        """[16, n] DRAM slice -> AP that reads it 8x (for the wrapped
        [128, n] gather-index layout)."""
        return bass.AP(src_ap.tensor, src_ap.offset,
                       [[0, 8]] + [list(d) for d in src_ap.ap])

    with TileContext(nc) as tc:
        with tc.tile_pool(name="const", bufs=1) as cpool, \
             tc.tile_pool(name="gath", bufs=2) as gpool, \
             tc.tile_pool(name="work", bufs=2) as wpool, \
             tc.tile_pool(name="small", bufs=3) as spool, \
             tc.tile_pool(name="psum", bufs=2, space="PSUM") as pspool, \
             tc.tile_pool(name="psc", bufs=1, space="PSUM") as pscpool, \
             tc.tile_pool(name="dram", bufs=1, space="DRAM") as dpool:

            # ---- device-side table setup ----
            if n_devices > 1:
                gsh_b = dpool.tile([D_CORE, RG], dt.float16)
                Gfull = dpool.tile([N_NODES, RG], dt.float16,
                                   addr_space="Shared")
                nc.gpsimd.dma_start(out=gsh_b[:], in_=Gshp[:])
                nc.gpsimd.collective_compute(
                    "AllGather", Alu.bypass,
                    replica_groups=[list(range(n_devices))],
                    ins=[gsh_b[:].opt()], outs=[Gfull[:].opt()])
            else:
                Gfull = Gshp  # full table passed directly (sim validation)

            # ---- constants ----
            iota = cpool.tile([P, P], dt.float32)
            nc.gpsimd.iota(iota[:], pattern=[[1, P]], base=0, channel_multiplier=0,
                           allow_small_or_imprecise_dtypes=True)
            iota16 = cpool.tile([P, P], dt.float16)
            nc.vector.tensor_copy(out=iota16[:], in_=iota[:])
            ones = cpool.tile([1, P], dt.float32)
            nc.vector.memset(ones[:], 1.0)
            scat_all = cpool.tile([P, B], dt.int32)
            nc.sync.dma_start(out=scat_all[:], in_=scat[:])

            tl = cpool.tile([1, 1], dt.float32)
            nc.sync.dma_start(out=tl[:], in_=lam0[:])
            tle = cpool.tile([1, 1], dt.float32)
            nc.scalar.activation(out=tle[:], in_=tl[:], func=Act.Exp)
            lps = pscpool.tile([P, 1], dt.float32)
            nc.tensor.matmul(out=lps[:], lhsT=ones[:], rhs=tle[:], start=True,
                             stop=True)
            lamb = cpool.tile([P, 1], dt.float32)
            nc.vector.tensor_copy(out=lamb[:], in_=lps[:])

            tp = cpool.tile([1, N_PATH], dt.float32)
            nc.sync.dma_start(out=tp[:], in_=pemb[:])
            tpe = cpool.tile([1, N_PATH], dt.float32)
            nc.scalar.activation(out=tpe[:], in_=tp[:], func=Act.Exp)
            nc.vector.tensor_scalar(out=tpe[:], in0=tpe[:], scalar1=5.0,
                                    scalar2=None, op0=Alu.min)
            pps = pscpool.tile([P, N_PATH], dt.float32)
            nc.tensor.matmul(out=pps[:], lhsT=ones[:], rhs=tpe[:], start=True,
                             stop=True)
            w1tab = cpool.tile([P, N_PATH], dt.float32)
            nc.vector.tensor_copy(out=w1tab[:], in_=pps[:])

            for bt in range(NBATCH):
                # ---- gathers for this batch (BB blocks) ----
                Gg = gpool.tile([P, BB * T, RG], dt.float16, tag="Gg")
                Ug = gpool.tile([P, BB * T, RU], dt.float16, tag="Ug")
                uit = spool.tile([P, NIU // 16], dt.int16, tag="uit")
                nc.sync.dma_start(out=uit[:], in_=bcast8(ui16[bt]))
                nc.gpsimd.dma_gather(Ug[:], Ushp[:], uit[:], NIU, NIU, RU,
                                     single_packet=False)
                for cch in range(NCHUNK):
                    git = spool.tile([P, NIG // 16], dt.int16, tag=f"git{cch}")
                    nc.sync.dma_start(
                        out=git[:],
                        in_=bcast8(gi16[bt, cch * 16:(cch + 1) * 16, :]))
                    dst = Gg[:, cch * (BB * TC):(cch + 1) * (BB * TC), :]
                    nc.gpsimd.dma_gather(dst,
                                         Gfull[cch * CHUNK:(cch + 1) * CHUNK, :],
                                         git[:], NIG, NIG, RG,
                                         single_packet=False)

                for bb in range(BB):
                    b = bt * BB + bb
                    dlc = spool.tile([P, T], dt.float16, tag="dlc")
                    nc.sync.dma_start(out=dlc[:], in_=dloc[b])
                    dlc32 = spool.tile([P, T], dt.float32, tag="dlc32")
                    nc.vector.tensor_copy(out=dlc32[:], in_=dlc[:])
                    ptt = spool.tile([P, T], dt.float16, tag="ptt")
                    nc.sync.dma_start(out=ptt[:], in_=ptl[b])

                    # 4-D views of this block's staging tiles:
                    # [P, cch(4), kk(TC), r]
                    Gap = Gg[:]
                    Gv = bass.AP(Gap.tensor, Gap.offset + bb * TC * RG,
                                 [Gap.ap[0], [BB * TC * RG, NCHUNK],
                                  [RG, TC], [1, RG]])
                    Uap = Ug[:]
                    Uv = bass.AP(Uap.tensor, Uap.offset + bb * TC * RU,
                                 [Uap.ap[0], [BB * TC * RU, NCHUNK],
                                  [RU, TC], [1, RU]])

                    prod = wpool.tile([P, T, RS], dt.float16, tag="prod")
                    prod4 = prod[:].rearrange("p (c k) r -> p c k r", c=NCHUNK)
                    nc.vector.tensor_tensor(out=prod4, in0=Uv[:, :, :, 0:RS],
                                            in1=Gv[:, :, :, 0:RS], op=Alu.mult)
                    red1 = spool.tile([P, T], dt.float32, tag="red1")
                    nc.vector.tensor_reduce(
                        out=red1[:], in_=prod[:, :, 0:HID],
                        axis=mybir.AxisListType.X, op=Alu.add)
                    red2 = spool.tile([P, T], dt.float32, tag="red2")
                    nc.vector.tensor_reduce(
                        out=red2[:], in_=prod[:, :, HID:RS],
                        axis=mybir.AxisListType.X, op=Alu.add)
                    score = spool.tile([P, T], dt.float32, tag="score")
                    nc.vector.tensor_scalar(out=score[:], in0=red1[:],
                                            scalar1=_INVSQRT, scalar2=None,
                                            op0=Alu.mult)
                    nc.vector.scalar_tensor_tensor(out=score[:], in0=red2[:],
                                                   scalar=lamb[:], in1=score[:],
                                                   op0=Alu.mult, op1=Alu.add)
                    nc.vector.tensor_scalar(out=score[:], in0=score[:],
                                            scalar1=2.0, scalar2=None,
                                            op0=Alu.min)
                    w0 = spool.tile([P, T], dt.float32, tag="w0")
                    nc.scalar.activation(out=w0[:], in_=score[:], func=Act.Exp)
                    nc.vector.tensor_scalar(out=w0[:], in0=w0[:], scalar1=5.0,
                                            scalar2=None, op0=Alu.min)
                    # per-slot path weight w1 = w1tab[pt] via 6 compares
                    tmp6 = wpool.tile([P, T, N_PATH], dt.float32, tag="tmp6")
                    for j in range(N_PATH):
                        nc.vector.tensor_scalar(out=tmp6[:, :, j], in0=ptt[:],
                                                scalar1=float(j),
                                                scalar2=w1tab[:, j:j + 1],
                                                op0=Alu.is_equal, op1=Alu.mult)
                    w1 = spool.tile([P, T], dt.float32, tag="w1")
                    nc.vector.tensor_reduce(out=w1[:], in_=tmp6[:],
                                            axis=mybir.AxisListType.X,
                                            op=Alu.add)

                    mw0 = wpool.tile([P, T, P], dt.float16, tag="mw0")
                    mw1 = wpool.tile([P, T, P], dt.float16, tag="mw1")
                    for t in range(T):
                        nc.vector.tensor_scalar(out=mw0[:, t, :], in0=iota16[:],
                                                scalar1=dlc32[:, t:t + 1],
                                                scalar2=w0[:, t:t + 1],
                                                op0=Alu.is_equal, op1=Alu.mult)
                        nc.vector.tensor_scalar(out=mw1[:, t, :], in0=iota16[:],
                                                scalar1=dlc32[:, t:t + 1],
                                                scalar2=w1[:, t:t + 1],
                                                op0=Alu.is_equal, op1=Alu.mult)
                    ps0 = pspool.tile([P, 1 + HID], dt.float32, tag="ps0")
                    ps1 = pspool.tile([P, 1 + HID], dt.float32, tag="ps1")
                    for t in range(T):
                        rhs = Gv[:, t // TC, t % TC, ONES_COL:ONES_COL + 1 + HID]
                        nc.tensor.matmul(out=ps0[:], lhsT=mw0[:, t, :], rhs=rhs,
                                         start=(t == 0), stop=(t == T - 1))
                    for t in range(T):
                        rhs = Gv[:, t // TC, t % TC, ONES_COL:ONES_COL + 1 + HID]
                        nc.tensor.matmul(out=ps1[:], lhsT=mw1[:, t, :], rhs=rhs,
                                         start=(t == 0), stop=(t == T - 1))

                    obuf = wpool.tile([P, HID], dt.float32, tag="obuf")
                    o1 = wpool.tile([P, HID], dt.float32, tag="o1")
                    for ps, dest in ((ps0, obuf), (ps1, o1)):
                        dz = spool.tile([P, 1], dt.float32, tag="dz")
                        nc.vector.tensor_scalar(out=dz[:], in0=ps[:, 0:1],
                                                scalar1=0.0, scalar2=None,
                                                op0=Alu.is_equal)
                        nc.vector.tensor_tensor(out=dz[:], in0=dz[:],
                                                in1=ps[:, 0:1], op=Alu.add)
                        nc.vector.tensor_scalar(out=dz[:], in0=dz[:], scalar1=2.0,
                                                scalar2=None, op0=Alu.mult)
                        rcp = spool.tile([P, 1], dt.float32, tag="rcp")
                        nc.vector.reciprocal(rcp[:], dz[:])
                        nc.scalar.activation(out=dest[:], in_=ps[:, 1:1 + HID],
                                             func=Act.Copy, scale=rcp[:])
                    nc.vector.tensor_tensor(out=obuf[:], in0=obuf[:], in1=o1[:],
                                            op=Alu.add)
                    nc.gpsimd.indirect_dma_start(
                        out=out[:],
                        out_offset=bass.IndirectOffsetOnAxis(
                            ap=scat_all[:, b:b + 1], axis=0),
                        in_=obuf[:], in_offset=None,
                        bounds_check=D_CORE - 1, oob_is_err=False)

    nc.finalize()
    return nc


_CACHE = {}


def _get_nc(B):
    if B not in _CACHE:
        _CACHE[B] = _build_bass(B)
    return _CACHE[B]


def run(inputs, trace=False):
    from concourse.bass_utils import run_bass_kernel_spmd
    per_core, B = _prepare(**inputs)
    nc = _get_nc(B)
    res = run_bass_kernel_spmd(nc, per_core, list(range(NCORES)), trace=trace)
    outs = [np.asarray(res.results[c]["out"]) for c in range(NCORES)]
    full = np.concatenate(outs, axis=0)
    return full, res


def kernel(**inputs):
    full, _ = run(inputs, trace=False)
    return full


# revision 12
# speedup vs baseline: 2.6002x; 1.1736x over previous
"""Trainium2 Bass kernel for nn_Attention_19971597927194 (GNN message passing).

Destination-sharded, fp16, minimal-host-transfer design.

Wall-clock through the axon tunnel is dominated by a fixed dispatch floor,
a per-parameter overhead (~1 ms each), and shipping input buffers to the
device each call (~10 GB/s); on-device compute is ~1 ms. So the layout
minimizes TOTAL bytes shipped across the 8 cores AND the parameter count:
  - All inputs are packed into ONE int16 blob per core (plus a small int32
    scatter-index table): unpadded fp16 node tables, compact gather-index
    lists, per-slot dest-local ids / path types, lambda0/path_emb.
  - The source-node table G = [k|eigs|1|v] fp16 is shipped SHARDED
    (12500 unpadded 289-elem rows per core), restrided on device to
    384-elem (768B) rows with zeroed padding, AllGathered into a Shared
    DRAM tensor, and edge rows are gathered from it with dma_gather.
  - The dest-node table U = [q|eigs] fp16 (160-elem rows) is restrided to
    256-elem (512B) rows the same way for the per-edge U gather.
  - Gather index lists ship compact ([16, n/16]) and are replicated to the
    wrapped [128, n/16] layout on device with a stride-0 broadcast DMA.

Compute structure (per core, proven in the fp32 baseline):
  - Edges sorted by destination; each core owns 12500 consecutive dests.
    Dests are packed into blocks of <=128; per block, edges are gathered via
    dma_gather from the G table (4 chunks of 25000 rows; int16 index limit)
    and the U table. Scores and exp-weights feed is_equal*weight one-hot
    mask matmuls that accumulate [den|num] per destination in PSUM; softmax
    normalization is applied once per destination (it commutes with the
    segment sum). Output rows are scattered with an indirect DMA.
"""
import sys

sys.path.insert(0, '/opt/trn_rl_repo')

import numpy as np

N_NODES = 100000
N_EDGES = 1000000
HID = 128
EDIM = 32
N_PATH = 6
NCORES = 8
P = 128

RS = HID + EDIM           # 160: score width ([q|eigs] . [k|eigs])
RU = 256                  # U padded row: [q(128)|eigs(32)|pad(96)] fp16 = 512B
RUC = 160                 # U payload width (shipped unpadded)
RG = 384                  # G padded row: [k|eigs|1|v|pad(95)] fp16 = 768B
GPW = 289                 # G payload width (shipped unpadded)
ONES_COL = HID + EDIM     # 160: the constant-1 column in G
NCHUNK = 4                # G table split (int16 index limit)
TC = 3                    # tiles per chunk group per block
T = NCHUNK * TC           # 12 tiles (of 128 slots) per block
CAPC = TC * P             # 384: per-chunk slot capacity per block
BB = 2                    # blocks per gather batch
D_CORE = N_NODES // NCORES
CHUNK = N_NODES // NCHUNK  # 25000
NIG = BB * TC * P          # idxs per chunk gather call (768)
NIU = BB * T * P           # idxs per U gather call (3072)
SENT = 30000.0             # dloc sentinel for pad slots (never equals 0..127)

_INVSQRT = float(1.0 / np.sqrt(np.float32(HID)))


def _offsets(B, g_rows):
    """Blob layout (int16 element offsets)."""
    NBATCH = B // BB
    off = {}
    off['G'] = 0
    off['U'] = off['G'] + g_rows * GPW
    off['GI'] = off['U'] + D_CORE * RUC
    off['UI'] = off['GI'] + NBATCH * NCHUNK * 16 * (NIG // 16)
    off['DL'] = off['UI'] + NBATCH * 16 * (NIU // 16)
    off['PT'] = off['DL'] + B * P * T
    off['LAM'] = off['PT'] + B * P * T
    off['PEMB'] = off['LAM'] + 1
    off['TOT'] = off['PEMB'] + N_PATH
    return off


def _wrap16(lst):
    """Compact dma_gather index layout: [16, n/16] int16 (device replicates
    to the wrapped [128, n/16] layout with a broadcast DMA)."""
    n = lst.shape[0]
    assert n % 16 == 0
    return lst.reshape(n // 16, 16).T.astype(np.int16)


def _prepare(q, k, v, eigs, lambda0, path_emb, indices, path_type):
    q = np.asarray(q, dtype=np.float32)
    k = np.asarray(k, dtype=np.float32)
    v = np.asarray(v, dtype=np.float32)
    eigs = np.asarray(eigs, dtype=np.float32)
    lam16 = np.asarray(lambda0, dtype=np.float16).reshape(1)
    pemb16 = np.asarray(path_emb, dtype=np.float16).reshape(N_PATH)
    i0 = np.asarray(indices[0])
    i1 = np.asarray(indices[1])
    pt = np.asarray(path_type)

    Gt = np.zeros((N_NODES, GPW), dtype=np.float16)
    Gt[:, 0:HID] = k.astype(np.float16)
    Gt[:, HID:HID + EDIM] = eigs.astype(np.float16)
    Gt[:, ONES_COL] = np.float16(1.0)
    Gt[:, ONES_COL + 1:ONES_COL + 1 + HID] = v.astype(np.float16)
    Ut = np.zeros((N_NODES, RUC), dtype=np.float16)
    Ut[:, 0:HID] = q.astype(np.float16)
    Ut[:, HID:RUC] = eigs.astype(np.float16)

    order = np.argsort(i0, kind='stable')
    i0s = i0[order].astype(np.int64)
    i1s = i1[order].astype(np.int64)
    pts = pt[order].astype(np.int64)
    core_bounds = np.searchsorted(i0s, np.arange(NCORES + 1) * D_CORE)

    # ---- per-core block packing (consecutive dests; per-chunk slot quota) ----
    core_blocks = []
    for c in range(NCORES):
        lo, hi = core_bounds[c], core_bounds[c + 1]
        i0l = i0s[lo:hi] - c * D_CORE
        chl = (i1s[lo:hi] // CHUNK).astype(np.int64)
        deg_pc = np.zeros((D_CORE, NCHUNK), np.int64)
        np.add.at(deg_pc, (i0l, chl), 1)
        assert deg_pc.max() <= CAPC
        blocks = []
        ds = 0
        while ds < D_CORE:
            de = ds
            cnt = np.zeros(NCHUNK, np.int64)
            while de < D_CORE and de - ds < P and np.all(cnt + deg_pc[de] <= CAPC):
                cnt += deg_pc[de]
                de += 1
            assert de > ds
            blocks.append((ds, de))
            ds = de
        core_blocks.append((lo, hi, blocks))
    B = max(len(b) for _, _, b in core_blocks)
    if B % BB:
        B += BB - (B % BB)
    NBATCH = B // BB
    off = _offsets(B, D_CORE)

    per_core = []
    for c in range(NCORES):
        lo, hi, blocks = core_blocks[c]
        i0l = i0s[lo:hi] - c * D_CORE
        i1c = i1s[lo:hi]
        ptc = pts[lo:hi]
        csum = np.concatenate([[0], np.cumsum(np.bincount(i0l, minlength=D_CORE))])

        gi16 = np.zeros((NBATCH, NCHUNK * 16, NIG // 16), dtype=np.int16)
        ui16 = np.zeros((NBATCH, 16, NIU // 16), dtype=np.int16)
        dloc = np.full((B, P, T), SENT, dtype=np.float16)
        ptl = np.zeros((B, P, T), dtype=np.float16)
        scat = np.full((B, P), 1 << 20, dtype=np.int32)

        gl_lists = np.zeros((NBATCH, NCHUNK, BB * TC * P), dtype=np.int64)
        ul_lists = np.zeros((NBATCH, BB * T * P), dtype=np.int64)

        for b, (ds, de) in enumerate(blocks):
            bt, bb = divmod(b, BB)
            e0, e1 = csum[ds], csum[de]
            sl = slice(e0, e1)
            ii0 = i0l[sl]
            ii1 = i1c[sl]
            ipt = ptc[sl]
            ch = ii1 // CHUNK
            # order edges by chunk, stable
            o2 = np.argsort(ch, kind='stable')
            ii0, ii1, ipt, ch = ii0[o2], ii1[o2], ipt[o2], ch[o2]
            cc = np.concatenate([[0], np.cumsum(np.bincount(ch, minlength=NCHUNK))])
            for cch in range(NCHUNK):
                g0, g1 = cc[cch], cc[cch + 1]
                n_g = g1 - g0
                assert n_g <= CAPC
                j = np.arange(n_g)
                kk = j // P          # tile within chunk group (0..TC-1)
                pp = j % P
                dloc[b, pp, cch * TC + kk] = (ii0[g0:g1] - ds).astype(np.float16)
                ptl[b, pp, cch * TC + kk] = ipt[g0:g1].astype(np.float16)
                gl_lists[bt, cch, bb * CAPC + j] = ii1[g0:g1] - cch * CHUNK
                # u index follows staging layout: staging tile s = cch*(BB*TC)
                # + bb*TC + kk, u-list position s*P + pp
                s_t = cch * (BB * TC) + bb * TC + kk
                ul_lists[bt, s_t * P + pp] = ii0[g0:g1]
            scat[b, 0:de - ds] = np.arange(ds, de, dtype=np.int32)

        for bt in range(NBATCH):
            for cch in range(NCHUNK):
                gi16[bt, cch * 16:(cch + 1) * 16, :] = _wrap16(gl_lists[bt, cch])
            ui16[bt] = _wrap16(ul_lists[bt])

        blob = np.zeros(off['TOT'], dtype=np.int16)
        blob[off['G']:off['U']] = \
            Gt[c * D_CORE:(c + 1) * D_CORE].view(np.int16).ravel()
        blob[off['U']:off['GI']] = \
            Ut[c * D_CORE:(c + 1) * D_CORE].view(np.int16).ravel()
        blob[off['GI']:off['UI']] = gi16.view(np.int16).ravel()
        blob[off['UI']:off['DL']] = ui16.view(np.int16).ravel()
        blob[off['DL']:off['PT']] = dloc.view(np.int16).ravel()
        blob[off['PT']:off['LAM']] = ptl.view(np.int16).ravel()
        blob[off['LAM']:off['PEMB']] = lam16.view(np.int16)
        blob[off['PEMB']:off['TOT']] = pemb16.view(np.int16)

        per_core.append(dict(blob=blob, scat=scat.transpose(1, 0).copy()))
    return per_core, B


def _build_bass(B, n_devices=NCORES):
    import concourse.bass as bass
    import concourse.bacc as bacc
    import concourse.mybir as mybir
    from concourse.tile import TileContext

    dt = mybir.dt
    Alu = mybir.AluOpType
    Act = mybir.ActivationFunctionType
    NBATCH = B // BB
    g_rows = D_CORE if n_devices > 1 else N_NODES
    off = _offsets(B, g_rows)

    nc = bacc.Bacc(None, num_devices=n_devices)
    blob = nc.declare_dram_parameter("blob", [off['TOT']], dt.int16,
                                     isOutput=False)
    scat = nc.declare_dram_parameter("scat", [P, B], dt.int32, isOutput=False)
    out = nc.declare_dram_parameter("out", [D_CORE, HID], dt.float32,
                                    isOutput=True)
    bap = blob[:]

    def bsl(o, dims, dtp=None):
        ap = bass.AP(bap.tensor, o, [list(d) for d in dims])
        return ap.bitcast(dtp) if dtp is not None else ap

    with TileContext(nc) as tc:
        with tc.tile_pool(name="const", bufs=1) as cpool, \
             tc.tile_pool(name="stage", bufs=2) as stpool, \
             tc.tile_pool(name="gath", bufs=2) as gpool, \
             tc.tile_pool(name="work", bufs=2) as wpool, \
             tc.tile_pool(name="small", bufs=3) as spool, \
             tc.tile_pool(name="psum", bufs=2, space="PSUM") as pspool, \
             tc.tile_pool(name="psc", bufs=1, space="PSUM") as pscpool, \
             tc.tile_pool(name="dram", bufs=1, space="DRAM") as dpool:

            # ---- device-side table setup: restride unpadded payload rows to
            # 256B-multiple rows (zero padding) via SBUF bounce tiles ----
            def restride(dst, src_off, rows, pw, rw):
                done = 0
                while done < rows:
                    n = min(P, rows - done)
                    sb = stpool.tile([P, rw], dt.float16, tag=f"rs{rw}")
                    nc.sync.dma_start(
                        out=sb[0:n, 0:pw],
                        in_=bsl(src_off + done * pw, [[pw, n], [1, pw]],
                                dt.float16))
                    nc.vector.memset(sb[0:n, pw:rw], 0.0)
                    nc.sync.dma_start(out=dst[done:done + n, :],
                                      in_=sb[0:n, :])
                    done += n

            upad = dpool.tile([D_CORE, RU], dt.float16)
            restride(upad, off['U'], D_CORE, RUC, RU)

            if n_devices > 1:
                gpad = dpool.tile([D_CORE, RG], dt.float16)
                restride(gpad, off['G'], D_CORE, GPW, RG)
                Gfull = dpool.tile([N_NODES, RG], dt.float16,
                                   addr_space="Shared")
                nc.gpsimd.collective_compute(
                    "AllGather", Alu.bypass,
                    replica_groups=[list(range(n_devices))],
                    ins=[gpad[:].opt()], outs=[Gfull[:].opt()])
            else:
                Gfull = dpool.tile([N_NODES, RG], dt.float16)
                restride(Gfull, off['G'], N_NODES, GPW, RG)

            # ---- constants ----
            iota = cpool.tile([P, P], dt.float32)
            nc.gpsimd.iota(iota[:], pattern=[[1, P]], base=0, channel_multiplier=0,
                           allow_small_or_imprecise_dtypes=True)
            iota16 = cpool.tile([P, P], dt.float16)
            nc.vector.tensor_copy(out=iota16[:], in_=iota[:])
            ones = cpool.tile([1, P], dt.float32)
            nc.vector.memset(ones[:], 1.0)
            scat_all = cpool.tile([P, B], dt.int32)
            nc.sync.dma_start(out=scat_all[:], in_=scat[:])

            tl16 = cpool.tile([1, 1], dt.float16)
            nc.sync.dma_start(out=tl16[:], in_=bsl(off['LAM'], [[1, 1], [1, 1]],
                                                   dt.float16))
            tl = cpool.tile([1, 1], dt.float32)
            nc.vector.tensor_copy(out=tl[:], in_=tl16[:])
            tle = cpool.tile([1, 1], dt.float32)
            nc.scalar.activation(out=tle[:], in_=tl[:], func=Act.Exp)
            lps = pscpool.tile([P, 1], dt.float32)
            nc.tensor.matmul(out=lps[:], lhsT=ones[:], rhs=tle[:], start=True,
                             stop=True)
            lamb = cpool.tile([P, 1], dt.float32)
            nc.vector.tensor_copy(out=lamb[:], in_=lps[:])

            tp16 = cpool.tile([1, N_PATH], dt.float16)
            nc.sync.dma_start(out=tp16[:],
                              in_=bsl(off['PEMB'], [[N_PATH, 1], [1, N_PATH]],
                                      dt.float16))
            tp = cpool.tile([1, N_PATH], dt.float32)
            nc.vector.tensor_copy(out=tp[:], in_=tp16[:])
            tpe = cpool.tile([1, N_PATH], dt.float32)
            nc.scalar.activation(out=tpe[:], in_=tp[:], func=Act.Exp)
            nc.vector.tensor_scalar(out=tpe[:], in0=tpe[:], scalar1=5.0,
                                    scalar2=None, op0=Alu.min)
            pps = pscpool.tile([P, N_PATH], dt.float32)
            nc.tensor.matmul(out=pps[:], lhsT=ones[:], rhs=tpe[:], start=True,
                             stop=True)
            w1tab = cpool.tile([P, N_PATH], dt.float32)
            nc.vector.tensor_copy(out=w1tab[:], in_=pps[:])

            for bt in range(NBATCH):
                # ---- gathers for this batch (BB blocks) ----
                Gg = gpool.tile([P, BB * T, RG], dt.float16, tag="Gg")
                Ug = gpool.tile([P, BB * T, RU], dt.float16, tag="Ug")
                uit = spool.tile([P, NIU // 16], dt.int16, tag="uit")
                nc.sync.dma_start(
                    out=uit[:],
                    in_=bsl(off['UI'] + bt * 16 * (NIU // 16),
                            [[0, 8], [NIU // 16, 16], [1, NIU // 16]]))
                nc.gpsimd.dma_gather(Ug[:], upad[:], uit[:], NIU, NIU, RU,
                                     single_packet=False)
                for cch in range(NCHUNK):
                    git = spool.tile([P, NIG // 16], dt.int16, tag=f"git{cch}")
                    nc.sync.dma_start(
                        out=git[:],
                        in_=bsl(off['GI'] + (bt * NCHUNK + cch) * 16 * (NIG // 16),
                                [[0, 8], [NIG // 16, 16], [1, NIG // 16]]))
                    dst = Gg[:, cch * (BB * TC):(cch + 1) * (BB * TC), :]
                    nc.gpsimd.dma_gather(dst,
                                         Gfull[cch * CHUNK:(cch + 1) * CHUNK, :],
                                         git[:], NIG, NIG, RG,
                                         single_packet=False)

                for bb in range(BB):
                    b = bt * BB + bb
                    dlc = spool.tile([P, T], dt.float16, tag="dlc")
                    nc.sync.dma_start(
                        out=dlc[:],
                        in_=bsl(off['DL'] + b * P * T, [[T, P], [1, T]],
                                dt.float16))
                    dlc32 = spool.tile([P, T], dt.float32, tag="dlc32")
                    nc.vector.tensor_copy(out=dlc32[:], in_=dlc[:])
                    ptt = spool.tile([P, T], dt.float16, tag="ptt")
                    nc.sync.dma_start(
                        out=ptt[:],
                        in_=bsl(off['PT'] + b * P * T, [[T, P], [1, T]],
                                dt.float16))

                    # 4-D views of this block's staging tiles:
                    # [P, cch(4), kk(TC), r]
                    Gap = Gg[:]
                    Gv = bass.AP(Gap.tensor, Gap.offset + bb * TC * RG,
                                 [Gap.ap[0], [BB * TC * RG, NCHUNK],
                                  [RG, TC], [1, RG]])
                    Uap = Ug[:]
                    Uv = bass.AP(Uap.tensor, Uap.offset + bb * TC * RU,
                                 [Uap.ap[0], [BB * TC * RU, NCHUNK],
                                  [RU, TC], [1, RU]])

                    prod = wpool.tile([P, T, RS], dt.float16, tag="prod")
                    prod4 = prod[:].rearrange("p (c k) r -> p c k r", c=NCHUNK)
                    nc.vector.tensor_tensor(out=prod4, in0=Uv[:, :, :, 0:RS],
                                            in1=Gv[:, :, :, 0:RS], op=Alu.mult)
                    red1 = spool.tile([P, T], dt.float32, tag="red1")
                    nc.vector.tensor_reduce(
                        out=red1[:], in_=prod[:, :, 0:HID],
                        axis=mybir.AxisListType.X, op=Alu.add)
                    red2 = spool.tile([P, T], dt.float32, tag="red2")
                    nc.vector.tensor_reduce(
                        out=red2[:], in_=prod[:, :, HID:RS],
                        axis=mybir.AxisListType.X, op=Alu.add)
                    score = spool.tile([P, T], dt.float32, tag="score")
                    nc.vector.tensor_scalar(out=score[:], in0=red1[:],
                                            scalar1=_INVSQRT, scalar2=None,
                                            op0=Alu.mult)
                    nc.vector.scalar_tensor_tensor(out=score[:], in0=red2[:],
                                                   scalar=lamb[:], in1=score[:],
                                                   op0=Alu.mult, op1=Alu.add)
                    nc.vector.tensor_scalar(out=score[:], in0=score[:],
                                            scalar1=2.0, scalar2=None,
                                            op0=Alu.min)
                    w0 = spool.tile([P, T], dt.float32, tag="w0")
                    nc.scalar.activation(out=w0[:], in_=score[:], func=Act.Exp)
                    nc.vector.tensor_scalar(out=w0[:], in0=w0[:], scalar1=5.0,
                                            scalar2=None, op0=Alu.min)

                    # per-slot path weight w1 = w1tab[pt] via 6 compares
                    tmp6 = wpool.tile([P, T, N_PATH], dt.float32, tag="tmp6")
                    for j in range(N_PATH):
                        nc.vector.tensor_scalar(out=tmp6[:, :, j], in0=ptt[:],
                                                scalar1=float(j),
                                                scalar2=w1tab[:, j:j + 1],
                                                op0=Alu.is_equal, op1=Alu.mult)
                    w1 = spool.tile([P, T], dt.float32, tag="w1")
                    nc.vector.tensor_reduce(out=w1[:], in_=tmp6[:],
                                            axis=mybir.AxisListType.X,
                                            op=Alu.add)

                    mw0 = wpool.tile([P, T, P], dt.float16, tag="mw0")
                    mw1 = wpool.tile([P, T, P], dt.float16, tag="mw1")
                    for t in range(T):
                        nc.vector.tensor_scalar(out=mw0[:, t, :], in0=iota16[:],
                                                scalar1=dlc32[:, t:t + 1],
                                                scalar2=w0[:, t:t + 1],
                                                op0=Alu.is_equal, op1=Alu.mult)
                        nc.vector.tensor_scalar(out=mw1[:, t, :], in0=iota16[:],
                                                scalar1=dlc32[:, t:t + 1],
                                                scalar2=w1[:, t:t + 1],
                                                op0=Alu.is_equal, op1=Alu.mult)
                    ps0 = pspool.tile([P, 1 + HID], dt.float32, tag="ps0")
                    ps1 = pspool.tile([P, 1 + HID], dt.float32, tag="ps1")
                    for t in range(T):
                        rhs = Gv[:, t // TC, t % TC, ONES_COL:ONES_COL + 1 + HID]
                        nc.tensor.matmul(out=ps0[:], lhsT=mw0[:, t, :], rhs=rhs,
                                         start=(t == 0), stop=(t == T - 1))
                    for t in range(T):
                        rhs = Gv[:, t // TC, t % TC, ONES_COL:ONES_COL + 1 + HID]
                        nc.tensor.matmul(out=ps1[:], lhsT=mw1[:, t, :], rhs=rhs,
                                         start=(t == 0), stop=(t == T - 1))

                    obuf = wpool.tile([P, HID], dt.float32, tag="obuf")
                    o1 = wpool.tile([P, HID], dt.float32, tag="o1")
                    for ps, dest in ((ps0, obuf), (ps1, o1)):
                        dz = spool.tile([P, 1], dt.float32, tag="dz")
                        nc.vector.tensor_scalar(out=dz[:], in0=ps[:, 0:1],
                                                scalar1=0.0, scalar2=None,
                                                op0=Alu.is_equal)
                        nc.vector.tensor_tensor(out=dz[:], in0=dz[:],
                                                in1=ps[:, 0:1], op=Alu.add)
                        nc.vector.tensor_scalar(out=dz[:], in0=dz[:], scalar1=2.0,
                                                scalar2=None, op0=Alu.mult)
                        rcp = spool.tile([P, 1], dt.float32, tag="rcp")
                        nc.vector.reciprocal(rcp[:], dz[:])
                        nc.scalar.activation(out=dest[:], in_=ps[:, 1:1 + HID],
                                             func=Act.Copy, scale=rcp[:])
                    nc.vector.tensor_tensor(out=obuf[:], in0=obuf[:], in1=o1[:],
                                            op=Alu.add)
                    nc.gpsimd.indirect_dma_start(
                        out=out[:],
                        out_offset=bass.IndirectOffsetOnAxis(
                            ap=scat_all[:, b:b + 1], axis=0),
                        in_=obuf[:], in_offset=None,
                        bounds_check=D_CORE - 1, oob_is_err=False)

    nc.finalize()
    return nc


_CACHE = {}


def _get_nc(B):
    if B not in _CACHE:
        _CACHE[B] = _build_bass(B)
    return _CACHE[B]


def run(inputs, trace=False):
    from concourse.bass_utils import run_bass_kernel_spmd
    per_core, B = _prepare(**inputs)
    nc = _get_nc(B)
    res = run_bass_kernel_spmd(nc, per_core, list(range(NCORES)), trace=trace)
    outs = [np.asarray(res.results[c]["out"]) for c in range(NCORES)]
    full = np.concatenate(outs, axis=0)
    return full, res


def kernel(**inputs):
    full, _ = run(inputs, trace=False)
    return full
